# revision 4
# baseline (speedup 1.0000x reference)
"""Self-contained Trainium2 Bass kernel for the EdgeNetwork GNN problem.

kernel(**inputs) takes the FULL unsharded inputs and returns the FULL
[100000, 32] output.

Strategy: shard by DESTINATION node range across 8 cores (no collectives).
Host routes each edge to the core owning its dst, sorts by dst, and packs
edges into 512-edge chunks (max 128 distinct dst "runs" per chunk, no run
crosses a chunk boundary).  Per chunk the device computes

    U^T[(k,j), n] = sum_e S[e, n] * ea[e, k] * x[e, j]      (PE matmuls)
    out[n, i]     = sum_{k,j} U^T[(k,j), n] * B[(k,j), i]   (PE matmuls)

where S[e, n] = 1 iff edge e belongs to the chunk's n-th dst run.  S is
materialised by an indirect-DMA gather of identity-matrix rows (row 128 is
all zeros, used by padding edges), so no vector-engine work is needed for
the segment sum at all.  The only DVE work is the Khatri-Rao product
Z[e, (k,j)] = ea[e,k] * x[e,j], one broadcast tensor_tensor per chunk.
Gathers/scatters are batched per 4096-edge superstep to amortise SWDGE
descriptor generation.  All PE traffic is fp16 (1 cycle/row); accumulation
happens in fp32 PSUM.
"""

import os
import sys
from contextlib import ExitStack

import numpy as np

for _p in ("/opt/trn_rl_repo", "/root/.axon_site/_ro/trn_rl_repo"):
    if os.path.isdir(_p) and _p not in sys.path:
        sys.path.insert(0, _p)

import concourse.mybir as mybir
import concourse.tile as tile
from concourse import bacc
from concourse.bass import IndirectOffsetOnAxis
from concourse.bass_utils import run_bass_kernel_spmd

N_NODES = 100000
D = 32
KE = 16
NCORES = 8
NPC = N_NODES // NCORES
CHUNK = 512          # edges per chunk (4 tiles of 128)
NRUNS = 128          # max dst runs per chunk (S-matrix columns)
SUPER = 4096         # edges per superstep (8 chunks)
SENTINEL = 16384     # scatter offset for unused run slots (> NPC-1)

F32 = mybir.dt.float32
F16 = mybir.dt.float16
I32 = mybir.dt.int32


# ---------------------------------------------------------------- host prep

def _pack_core_edges(dst_sorted_idx, dst_local):
    """Pack dst-sorted edges into chunks of <=CHUNK edges and <=NRUNS nodes.

    Every local node id (including zero-degree nodes) occupies exactly one
    run slot in exactly one chunk, so every output row is written exactly
    once by the scatter.  Nodes per chunk are CONSECUTIVE ids, which makes
    slot/sidx generation fully vectorised.

    Returns (order, slot, sidx):
      order [n_chunks*CHUNK] int64: edge id per packed position (-1 = pad)
      slot  [n_chunks*CHUNK] int32: run index within chunk (NRUNS = pad)
      sidx  [n_chunks, NRUNS] int32: local dst node per run (SENTINEL = unused)
    """
    lengths = np.bincount(dst_local, minlength=NPC).astype(np.int64)
    assert lengths.max(initial=0) <= CHUNK, "single dst exceeds chunk capacity"
    cum = np.concatenate([[0], np.cumsum(lengths)])   # edges before node i

    # greedy chunk cuts over consecutive node ids
    cuts = [0]
    i = 0
    while i < NPC:
        j = min(i + NRUNS, NPC)
        j2 = int(np.searchsorted(cum, cum[i] + CHUNK, side="right")) - 1
        j = min(j, j2)
        assert j > i
        cuts.append(j)
        i = j
    cuts = np.asarray(cuts, dtype=np.int64)
    nch = len(cuts) - 1
    ch_node0, ch_node1 = cuts[:-1], cuts[1:]
    ch_e0 = cum[ch_node0]                 # first edge (sorted order) per chunk
    ch_cnt = cum[ch_node1] - ch_e0        # edges per chunk

    n = len(dst_sorted_idx)
    # chunk of each sorted edge, and its packed position
    e_chunk = np.searchsorted(ch_e0, np.arange(n), side="right") - 1
    pos = e_chunk * CHUNK + (np.arange(n) - ch_e0[e_chunk])

    order = np.full(nch * CHUNK, -1, np.int64)
    order[pos] = dst_sorted_idx
    slot = np.full(nch * CHUNK, NRUNS, np.int32)
    slot[pos] = dst_local - ch_node0[e_chunk]

    sidx = np.full((nch, NRUNS), SENTINEL, np.int32)
    r = np.arange(NRUNS)[None, :]
    node_ids = ch_node0[:, None] + r
    valid = r < (ch_node1 - ch_node0)[:, None]
    sidx[valid] = node_ids[valid]
    return order, slot, sidx


def _prepare(node_attr, edge_attr, pair_indices, kernel, bias):
    dst = np.asarray(pair_indices[:, 0], dtype=np.int64)
    src = np.asarray(pair_indices[:, 1], dtype=np.int64)
    ea = np.asarray(edge_attr, dtype=np.float32)
    kern = np.asarray(kernel, dtype=np.float32)
    bias = np.asarray(bias, dtype=np.float32)

    use_bias = bool(np.any(bias != 0.0))
    if use_bias:
        KP = KE + 1
        kern_full = np.concatenate([kern, bias[None, :]], axis=0)
    else:
        KP = KE
        kern_full = kern
    KG = (KP + 3) // 4
    KPAD = KG * 4

    # B[(k,j), i] = kern[k, i*D + j], zero-padded to KPAD k's
    B = np.zeros((KPAD * D, D), dtype=np.float32)
    Bk = kern_full.reshape(KP, D, D).transpose(0, 2, 1)   # [KP, j, i]
    B[: KP * D] = Bk.reshape(KP * D, D)

    # identity gather table: row s = e_s for s < NRUNS, row NRUNS = zeros
    IDENT = np.zeros((NRUNS + 1, NRUNS), dtype=np.float16)
    IDENT[:NRUNS, :NRUNS] = np.eye(NRUNS, dtype=np.float16)

    per_core_raw = []
    max_chunks = 0
    for c in range(NCORES):
        lo, hi = c * NPC, (c + 1) * NPC
        sel = np.nonzero((dst >= lo) & (dst < hi))[0]
        d_loc_unsorted = dst[sel] - lo
        s_ord = np.argsort(d_loc_unsorted, kind="stable")
        order, slot, sidx = _pack_core_edges(sel[s_ord],
                                             d_loc_unsorted[s_ord])
        per_core_raw.append((order, slot, sidx))
        max_chunks = max(max_chunks, len(sidx))

    NSUP = (max_chunks + 7) // 8
    NCH = NSUP * 8
    Epad = NCH * CHUNK

    def swz(a):
        # [NSUP*8*4*128, ...] -> [NSUP, 128, 8*4, ...] (col = q*4 + t)
        a = a.reshape(NSUP, 8, 4, 128, *a.shape[1:])
        return np.ascontiguousarray(np.moveaxis(a, 3, 1))

    per_core = []
    node_f16 = np.ascontiguousarray(node_attr, dtype=np.float16)
    for c in range(NCORES):
        order, slot, sidx = per_core_raw[c]
        nch = len(sidx)
        order = np.concatenate([order, np.full((NCH - nch) * CHUNK, -1,
                                               np.int64)])
        slot = np.concatenate([slot, np.full((NCH - nch) * CHUNK, NRUNS,
                                             np.int32)])
        sidx = np.concatenate([sidx, np.full((NCH - nch, NRUNS), SENTINEL,
                                             np.int32)])

        real = order >= 0
        oe = np.where(real, order, 0)

        eaP = np.zeros((Epad, KPAD), dtype=np.float16)
        eaP[real, :KE] = ea[oe[real]].astype(np.float16)
        if use_bias:
            eaP[real, KE] = 1.0
        srcP = np.where(real, src[oe], 0).astype(np.int32)

        # sidx: [NCH, NRUNS] -> [NSUP, 8, 128] -> [NSUP, 128, 8]
        sidx_sw = np.ascontiguousarray(
            sidx.reshape(NSUP, 8, NRUNS).transpose(0, 2, 1))

        per_core.append(dict(
            ea_sw=swz(eaP).reshape(NSUP, 128, 32 * KPAD),
            src_sw=swz(srcP).reshape(NSUP, 128, 32),
            slot_sw=swz(slot).reshape(NSUP, 128, 32),
            sidx_sw=sidx_sw,
            node_attr=node_f16,
            B=B.astype(np.float16),
            IDENT=IDENT,
        ))
    meta = dict(Epad=Epad, NSUP=NSUP, KG=KG, KPAD=KPAD)
    return per_core, meta


# ------------------------------------------------------------- bass program

def _build(NSUP, KPAD, KG, POOL_Z_PERIOD=2):
    """POOL_Z_PERIOD: every POOL_Z_PERIOD-th chunk offloads one of its four
    Z-build tiles to the GPSIMD (Pool) engine to relieve the DVE bottleneck
    (0 = never)."""
    nc = bacc.Bacc("TRN2", target_bir_lowering=False, debug=False)

    KJ = KPAD * D            # Khatri-Rao width (kj columns), KG blocks of 128
    ea_d = nc.dram_tensor("ea_sw", [NSUP, 128, 32 * KPAD], F16,
                          kind="ExternalInput").ap()
    src_d = nc.dram_tensor("src_sw", [NSUP, 128, 32], I32,
                           kind="ExternalInput").ap()
    slot_d = nc.dram_tensor("slot_sw", [NSUP, 128, 32], I32,
                            kind="ExternalInput").ap()
    sidx_d = nc.dram_tensor("sidx_sw", [NSUP, 128, 8], I32,
                            kind="ExternalInput").ap()
    node_d = nc.dram_tensor("node_attr", [N_NODES, D], F16,
                            kind="ExternalInput").ap()
    b_d = nc.dram_tensor("B", [KJ, D], F16, kind="ExternalInput").ap()
    id_d = nc.dram_tensor("IDENT", [NRUNS + 1, NRUNS], F16,
                          kind="ExternalInput").ap()
    out_d = nc.dram_tensor("out", [NPC, D], F32, kind="ExternalOutput").ap()

    with tile.TileContext(nc) as tc, ExitStack() as ctx:
        const_pool = ctx.enter_context(tc.tile_pool(name="const", bufs=1))
        sup_pool = ctx.enter_context(tc.tile_pool(name="sup", bufs=2))
        z_pool = ctx.enter_context(tc.tile_pool(name="z", bufs=3))
        ut_pool = ctx.enter_context(tc.tile_pool(name="ut", bufs=3))
        ot_pool = ctx.enter_context(tc.tile_pool(name="ot", bufs=2))
        put_pool = ctx.enter_context(
            tc.tile_pool(name="put", bufs=3, space="PSUM"))
        po_pool = ctx.enter_context(
            tc.tile_pool(name="po", bufs=2, space="PSUM"))

        b_sb = const_pool.tile([128, KG * D], F16, tag="b")
        for g in range(KG):
            nc.sync.dma_start(b_sb[:, g * D:(g + 1) * D],
                              b_d[g * 128:(g + 1) * 128, :])

        for s in range(NSUP):
            ea_sb = sup_pool.tile([128, 32 * KPAD], F16, tag="ea")
            nc.sync.dma_start(ea_sb[:], ea_d[s])
            src_sb = sup_pool.tile([128, 32], I32, tag="src")
            nc.sync.dma_start(src_sb[:], src_d[s])
            slot_sb = sup_pool.tile([128, 32], I32, tag="slot")
            nc.sync.dma_start(slot_sb[:], slot_d[s])
            sidx_sb = sup_pool.tile([128, 8], I32, tag="sidx")
            nc.sync.dma_start(sidx_sb[:], sidx_d[s])

            # batched gathers: x rows and S (identity) rows for all 32 tiles
            x_sb = sup_pool.tile([128, 32 * D], F16, tag="x")
            nc.gpsimd.indirect_dma_start(
                out=x_sb[:], out_offset=None, in_=node_d[:],
                in_offset=IndirectOffsetOnAxis(ap=src_sb[:], axis=0))
            s_sb = sup_pool.tile([128, 32 * NRUNS], F16, tag="s")
            nc.gpsimd.indirect_dma_start(
                out=s_sb[:], out_offset=None, in_=id_d[:],
                in_offset=IndirectOffsetOnAxis(ap=slot_sb[:], axis=0))

            ot = ot_pool.tile([128, 8 * D], F32, tag="ot")

            for q in range(8):
                # Z[e, (t,k,j)] = ea[e, (t,k)] * x[e, (t,j)] for whole chunk
                z_t = z_pool.tile([128, 4 * KJ], F16, tag="z")
                x_b = x_sb[:, q * 4 * D:(q + 1) * 4 * D] \
                    .rearrange("p (t o j) -> p t o j", t=4, o=1) \
                    .to_broadcast([128, 4, KPAD, D])
                ea_b = ea_sb[:, q * 4 * KPAD:(q + 1) * 4 * KPAD] \
                    .rearrange("p (t k o) -> p t k o", t=4, o=1) \
                    .to_broadcast([128, 4, KPAD, D])
                nc.vector.tensor_tensor(
                    out=z_t[:].rearrange("p (t k j) -> p t k j", t=4, j=D),
                    in0=x_b, in1=ea_b, op=mybir.AluOpType.mult)

                # UT[(kj), n] += Z[e, kj]^T @ S[e, n]  (contract edges)
                ut_ps = put_pool.tile([128, KG * NRUNS], F32, tag="utp")
                for g in range(KG):
                    for t in range(4):
                        nc.tensor.matmul(
                            out=ut_ps[:, g * NRUNS:(g + 1) * NRUNS],
                            lhsT=z_t[:, t * KJ + g * 128:t * KJ + (g + 1) * 128],
                            rhs=s_sb[:, (q * 4 + t) * NRUNS:
                                     (q * 4 + t + 1) * NRUNS],
                            start=(t == 0), stop=(t == 3))

                ut_sb = ut_pool.tile([128, KG * NRUNS], F16, tag="uts")
                nc.scalar.copy(out=ut_sb[:], in_=ut_ps[:])

                # out[n, i] = sum_g UT_g[kj, n]^T @ B_g[kj, i]
                po = po_pool.tile([128, D], F32, tag="po")
                for g in range(KG):
                    nc.tensor.matmul(
                        out=po[:],
                        lhsT=ut_sb[:, g * NRUNS:(g + 1) * NRUNS],
                        rhs=b_sb[:, g * D:(g + 1) * D],
                        start=(g == 0), stop=(g == KG - 1))
                nc.scalar.copy(out=ot[:, q * D:(q + 1) * D], in_=po[:])

            # batched scatter: row (p, q) -> out_d[sidx[p, q]]
            nc.gpsimd.indirect_dma_start(
                out=out_d[:],
                out_offset=IndirectOffsetOnAxis(ap=sidx_sb[:], axis=0),
                in_=ot[:], in_offset=None,
                bounds_check=NPC - 1, oob_is_err=False)

    nc.compile()
    return nc


_CACHE = {}


def kernel(node_attr, edge_attr, pair_indices, kernel, bias):
    per_core, meta = _prepare(node_attr, edge_attr, pair_indices,
                              kernel, bias)
    key = (meta["NSUP"], meta["KPAD"], meta["KG"])
    if key not in _CACHE:
        _CACHE[key] = _build(*key)
    nc = _CACHE[key]
    res = run_bass_kernel_spmd(nc, per_core, list(range(NCORES)))
    out = np.concatenate([res.results[c]["out"] for c in range(NCORES)],
                         axis=0)
    return np.ascontiguousarray(out, dtype=np.float32)
